# revision 4
# baseline (speedup 1.0000x reference)
"""Prefix-LM causal self-attention on 8 TRN2 NeuronCores.

Strategy (tensor-parallel over heads):
  - 16 heads / 8 cores = 2 heads per core; each core handles its 2 heads for
    both batch elements (4 attention instances), computing in transposed
    layouts so no on-device transposes of x are needed:
      qT/kT = W.T @ x.T           [d, t]   (x.T supplied pre-transposed by host)
      vT -> v via PE transpose    [t, d]
      S^T  = kT.T-tile @ qT       [j, i]   (scores transposed, prefix-LM mask)
      att  = exp(S^T) (* mask)             (no max-subtraction: scores are O(1))
      O^T/l via [v | ones] matmul [d+1, i] (denominator rides along as row 64)
      yT   = O^T / l + bv         [d, t]
      outT = Wproj_rows.T @ yT    [m, t]   (partial over this core's 128 c-rows)
  - Host sums the 8 partial outT's and transposes back. All matmuls run as
    float32r (TF32-like, 4x faster than fp32 on TensorE, ~1e-4 rel err).
  - Mask sparsity: fully-masked 128x512 score tiles are skipped entirely
    (~37% of tiles at prefix_len=128); partial tiles get a 0/1 multiplicative
    mask (additive -inf is avoided; masked exp terms are exactly zeroed).
"""

import numpy as np

import concourse.bacc as bacc
import concourse.bass as bass
import concourse.mybir as mybir
import concourse.tile as tile
from concourse.bass_utils import run_bass_kernel_spmd

F32 = mybir.dt.float32
F32R = mybir.dt.float32r
AF = mybir.ActivationFunctionType

B, T, C, H, D = 2, 2048, 1024, 16, 64
TT = B * T            # 4096 (batch folded into time)
N_CORES = 8
IB = 512              # i-block (query columns per score tile)
JT = 128              # j-tile (key rows per score tile)
TBLK = 512            # t-block for projections
N_CB = C // 128       # 8 contraction blocks
N_TB = TT // TBLK     # 8
N_MT = C // 128       # 8 Wproj column tiles
N_IB = T // IB        # 4 i-blocks per batch element
N_JT = T // JT        # 16 j-tiles per batch element


def _plan(P):
    """Classify each (j-tile, i-block) score tile of the [T, T] mask.

    allowed(i, j) = (i < P) | (j < P) | (j <= i)   [negation of the reference's
    disallow = (i >= P) & (j >= P) & (j > i)]

    Returns (plan, patterns): plan[ib] = list of (jt, mask_id_or_None) for
    tiles that must be computed; patterns = float32 [n_pat, JT, IB] of 0/1.
    """
    patterns, pat_ids, plan = [], {}, []
    ii_base = np.arange(IB)[None, :]
    jj_base = np.arange(JT)[:, None]
    for ib in range(N_IB):
        i0 = ib * IB
        row = []
        for jt in range(N_JT):
            j0 = jt * JT
            ii = i0 + ii_base
            jj = j0 + jj_base
            allowed = (ii < P) | (jj < P) | (jj <= ii)
            if not allowed.any():
                continue
            if allowed.all():
                row.append((jt, None))
            else:
                m = allowed.astype(np.float32)
                key = m.tobytes()
                if key not in pat_ids:
                    pat_ids[key] = len(patterns)
                    patterns.append(m)
                row.append((jt, pat_ids[key]))
        plan.append(row)
    pats = np.stack(patterns) if patterns else np.zeros((0, JT, IB), np.float32)
    return plan, pats


def build(plan, n_pat, loop_n=1):
    nc = bacc.Bacc()

    xT_d = nc.dram_tensor("xT", [C, TT], F32, kind="ExternalInput")
    w3_d = nc.dram_tensor("w3", [C, 384], F32, kind="ExternalInput")
    wp_d = nc.dram_tensor("wp", [128, C], F32, kind="ExternalInput")
    bq_d = nc.dram_tensor("bq", [128, 1], F32, kind="ExternalInput")
    bk_d = nc.dram_tensor("bk", [128, 1], F32, kind="ExternalInput")
    bv_d = nc.dram_tensor("bv", [64, 2], F32, kind="ExternalInput")
    bp_d = nc.dram_tensor("bp", [128, N_MT], F32, kind="ExternalInput")
    if n_pat:
        mk_d = nc.dram_tensor("mk", [n_pat, JT, IB], F32, kind="ExternalInput")
    out_d = nc.dram_tensor("out", [C, TT], F32, kind="ExternalOutput")

    with tile.TileContext(nc) as tc:
        with (
            tc.tile_pool(name="const", bufs=1) as const,
            tc.tile_pool(name="big", bufs=1) as bigp,
            tc.tile_pool(name="xtp", bufs=2) as xtp,
            tc.tile_pool(name="vtsp", bufs=2) as vtsp,
            tc.tile_pool(name="attp", bufs=3) as attp,
            tc.tile_pool(name="smallp", bufs=2) as smallp,
            tc.tile_pool(name="outp", bufs=3) as outp,
            tc.tile_pool(name="ps_mm", bufs=2, space="PSUM") as ps_mm,
            tc.tile_pool(name="ps_s", bufs=2, space="PSUM") as ps_s,
            tc.tile_pool(name="ps_o", bufs=1, space="PSUM") as ps_o,
            tc.tile_pool(name="ps_vt", bufs=2, space="PSUM") as ps_vt,
        ):
            # ---- constants (loaded once, outside any timing loop) ----
            w3_sb = const.tile([128, N_CB, 384], F32R)
            nc.sync.dma_start(
                out=w3_sb,
                in_=w3_d.rearrange("(cb p) e -> p cb e", p=128).bitcast(F32R),
            )
            wp_sb = const.tile([128, N_MT, 128], F32R)
            nc.sync.dma_start(
                out=wp_sb,
                in_=wp_d.rearrange("p (mt m) -> p mt m", m=128).bitcast(F32R),
            )
            bq_sb = const.tile([128, 1], F32)
            nc.sync.dma_start(out=bq_sb, in_=bq_d[:, :])
            bk_sb = const.tile([128, 1], F32)
            nc.sync.dma_start(out=bk_sb, in_=bk_d[:, :])
            bv_sb = const.tile([64, 2], F32)
            nc.sync.dma_start(out=bv_sb, in_=bv_d[:, :])
            bp_sb = const.tile([128, N_MT], F32)
            nc.sync.dma_start(out=bp_sb, in_=bp_d[:, :])
            if n_pat:
                mk_sb = const.tile([128, n_pat, IB], F32R)
                nc.sync.dma_start(
                    out=mk_sb, in_=mk_d.rearrange("n p q -> p n q").bitcast(F32R)
                )
            # identity for PE transpose (f32r to match the data dtype)
            id_f32 = const.tile([128, 128], F32)
            nc.vector.memset(id_f32, 0.0)
            nc.gpsimd.affine_select(
                out=id_f32,
                in_=id_f32,
                compare_op=mybir.AluOpType.not_equal,
                fill=1.0,
                base=0,
                pattern=[[-1, 128]],
                channel_multiplier=1,
            )
            id_sb = const.tile([128, 128], F32R)
            nc.vector.tensor_copy(out=id_sb, in_=id_f32)

            ones_f32 = const.tile([128, TT // JT, 1], F32)
            nc.vector.memset(ones_f32, 1.0)

            qT2 = bigp.tile([128, TT], F32R)
            kT2 = bigp.tile([128, TT], F32R)
            vsb = bigp.tile([128, TT // JT, 130], F32R)
            yT = bigp.tile([128, TT], F32R)

            # ones columns for the [v | 1] denominator trick
            nc.vector.tensor_copy(out=vsb[:, :, 64:65], in_=ones_f32)
            nc.vector.tensor_copy(out=vsb[:, :, 129:130], in_=ones_f32)

            def body(_iv=None):

                # ---- Phase A: QKV projections (transposed layouts) ----
                for tb in range(N_TB):
                    ts_ = bass.ts(tb, TBLK)
                    xt = xtp.tile([128, N_CB, TBLK], F32R, tag="xt")
                    nc.sync.dma_start(
                        out=xt,
                        in_=xT_d.rearrange("(cb p) t -> p cb t", p=128)[
                            :, :, ts_
                        ].bitcast(F32R),
                    )
                    for part in range(3):  # q, k, v
                        pm = ps_mm.tile([128, TBLK], F32, tag="mm")
                        for cb in range(N_CB):
                            nc.tensor.matmul(
                                pm,
                                w3_sb[:, cb, bass.ts(part, 128)],
                                xt[:, cb, :],
                                start=(cb == 0),
                                stop=(cb == N_CB - 1),
                            )
                        if part == 0:
                            nc.scalar.activation(
                                out=qT2[:, ts_], in_=pm, func=AF.Identity,
                                bias=bq_sb, scale=0.125,
                            )
                        elif part == 1:
                            nc.scalar.activation(
                                out=kT2[:, ts_], in_=pm, func=AF.Identity,
                                bias=bk_sb, scale=1.0,
                            )
                        else:
                            vts = vtsp.tile([128, TBLK], F32R, tag="vts")
                            nc.vector.tensor_copy(out=vts, in_=pm)
                            for st in range(TBLK // 128):
                                pv = ps_vt.tile([128, 128], F32R, tag="vt")
                                nc.tensor.transpose(
                                    pv, vts[:, bass.ts(st, 128)], id_sb
                                )
                                jg = tb * (TBLK // 128) + st
                                nc.vector.tensor_copy(
                                    out=vsb[:, jg, 0:64], in_=pv[:, 0:64]
                                )
                                nc.vector.tensor_copy(
                                    out=vsb[:, jg, 65:129], in_=pv[:, 64:128]
                                )

                # ---- Phase B: attention per (batch, i-block), heads paired ----
                for b in range(B):
                    for ib in range(N_IB):
                        icols = bass.ds(b * T + ib * IB, IB)
                        tiles = plan[ib]
                        last = len(tiles) - 1
                        ot = [
                            ps_o.tile([65, IB], F32, tag=f"o{h}", name=f"ot{h}") for h in (0, 1)
                        ]
                        for idx, (jt, mid) in enumerate(tiles):
                            jcols = bass.ds(b * T + jt * JT, JT)
                            sps = []
                            for h in (0, 1):
                                sp = ps_s.tile([128, IB], F32, tag="s")
                                hd = bass.ds(h * 64, 64)
                                nc.tensor.matmul(
                                    sp, kT2[hd, jcols], qT2[hd, icols],
                                    start=True, stop=True,
                                )
                                sps.append(sp)
                            for h in (0, 1):
                                att = attp.tile([128, IB], F32R, tag="att")
                                nc.scalar.activation(
                                    out=att, in_=sps[h], func=AF.Exp, scale=1.0
                                )
                                if mid is not None:
                                    nc.vector.tensor_mul(
                                        out=att, in0=att, in1=mk_sb[:, mid, :]
                                    )
                                nc.tensor.matmul(
                                    ot[h],
                                    vsb[:, b * N_JT + jt, bass.ds(h * 65, 65)],
                                    att,
                                    start=(idx == 0),
                                    stop=(idx == last),
                                )
                        for h in (0, 1):
                            lst = smallp.tile([65, IB], F32, tag="lst")
                            nc.vector.tensor_copy(
                                out=lst[64:65, :], in_=ot[h][64:65, :]
                            )
                            l0 = smallp.tile([1, IB], F32, tag="l0")
                            nc.sync.dma_start(out=l0, in_=lst[64:65, :])
                            lb = smallp.tile([64, IB], F32, tag="lb")
                            nc.gpsimd.partition_broadcast(lb, l0)
                            nc.vector.reciprocal(lb, lb)
                            if h == 0:
                                nc.vector.tensor_mul(
                                    out=yT[0:64, icols], in0=ot[h][0:64, :], in1=lb
                                )
                                nc.vector.tensor_scalar_add(
                                    out=yT[0:64, icols],
                                    in0=yT[0:64, icols],
                                    scalar1=bv_sb[:, 0:1],
                                )
                            else:
                                yst = smallp.tile([64, IB], F32R, tag="yst")
                                nc.vector.tensor_mul(
                                    out=yst, in0=ot[h][0:64, :], in1=lb
                                )
                                nc.vector.tensor_scalar_add(
                                    out=yst, in0=yst, scalar1=bv_sb[:, 1:2]
                                )
                                nc.sync.dma_start(
                                    out=yT[64:128, icols], in_=yst
                                )

                # ---- Phase C: output projection (partial over our c-rows) ----
                for mt in range(N_MT):
                    for tb in range(N_TB):
                        ts_ = bass.ts(tb, TBLK)
                        pp = ps_mm.tile([128, TBLK], F32, tag="mm")
                        nc.tensor.matmul(
                            pp, wp_sb[:, mt, :], yT[:, ts_],
                            start=True, stop=True,
                        )
                        osb = outp.tile([128, TBLK], F32, tag="osb")
                        nc.vector.tensor_scalar_add(
                            out=osb, in0=pp, scalar1=bp_sb[:, mt : mt + 1]
                        )
                        nc.sync.dma_start(
                            out=out_d[bass.ts(mt, 128), ts_], in_=osb
                        )

            if loop_n == 1:
                body()
            else:
                with tc.For_i(0, loop_n, 1) as iv:
                    body(iv)

    nc.finalize()
    return nc


_CACHE = {}


def _get_nc(P, loop_n=1):
    key = (int(P), int(loop_n))
    if key not in _CACHE:
        plan, pats = _plan(int(P))
        _CACHE[key] = (build(plan, len(pats), loop_n), pats)
    return _CACHE[key]


def make_in_maps(x, Wqkv, bqkv, Wproj, bproj, pats):
    x = np.asarray(x, np.float32)
    Wqkv = np.asarray(Wqkv, np.float32)
    bqkv = np.asarray(bqkv, np.float32)
    Wproj = np.asarray(Wproj, np.float32)
    bproj = np.asarray(bproj, np.float32)
    xT = np.ascontiguousarray(x.reshape(TT, C).T)
    in_maps = []
    for i in range(N_CORES):
        cs = slice(128 * i, 128 * i + 128)
        w3 = np.ascontiguousarray(
            np.concatenate(
                [Wqkv[:, cs], Wqkv[:, 1024:2048][:, cs], Wqkv[:, 2048:3072][:, cs]],
                axis=1,
            )
        )
        m = {
            "xT": xT,
            "w3": w3,
            "wp": np.ascontiguousarray(Wproj[cs, :]),
            "bq": np.ascontiguousarray((bqkv[0:1024][cs] / 8.0).reshape(128, 1)),
            "bk": np.ascontiguousarray(bqkv[1024:2048][cs].reshape(128, 1)),
            "bv": np.ascontiguousarray(
                bqkv[2048:3072][cs].reshape(2, 64).T
            ),
            "bp": (
                np.ascontiguousarray(bproj.reshape(N_MT, 128).T)
                if i == 0
                else np.zeros((128, N_MT), np.float32)
            ),
        }
        if len(pats):
            m["mk"] = pats
        in_maps.append(m)
    return in_maps


def kernel(x, Wqkv, bqkv, Wproj, bproj, prefix_len):
    P = int(prefix_len)
    nc, pats = _get_nc(P)
    in_maps = make_in_maps(x, Wqkv, bqkv, Wproj, bproj, pats)
    res = run_bass_kernel_spmd(nc, in_maps, core_ids=list(range(N_CORES)))
    acc = np.zeros((C, TT), np.float32)
    for i in range(N_CORES):
        acc += res.results[i]["out"]
    return np.ascontiguousarray(acc.T).reshape(B, T, C)


# revision 21
# speedup vs baseline: 16.7941x; 16.7941x over previous
"""Prefix-LM causal self-attention on 8 TRN2 NeuronCores.

Strategy (tensor-parallel over heads):
  - 16 heads / 8 cores = 2 heads per core; each core handles its 2 heads for
    both batch elements (4 attention instances), computing in transposed
    layouts so no on-device transposes of x are needed:
      qT/kT = W.T @ x.T           [d, t]   (x.T supplied pre-transposed by host)
      vT -> v via PE transpose    [t, d]
      S^T  = kT.T-tile @ qT       [j, i]   (scores transposed, prefix-LM mask)
      att  = exp(S^T) (* mask)             (no max-subtraction: scores are O(1))
      O^T/l via [v | ones] matmul [d+1, i] (denominator rides along as row 64)
      yT   = O^T / l + bv         [d, t]
      outT = Wproj_rows.T @ yT    [m, t]   (partial over this core's 128 c-rows)
  - Host sums the 8 partial outT's and transposes back. All matmuls run as
    float32r (TF32-like, 4x faster than fp32 on TensorE, ~1e-4 rel err).
  - Mask sparsity: fully-masked 128x512 score tiles are skipped entirely
    (~37% of tiles at prefix_len=128); partial tiles get a 0/1 multiplicative
    mask (additive -inf is avoided; masked exp terms are exactly zeroed).
"""

import numpy as np

import concourse.bacc as bacc
import concourse.bass as bass
import concourse.mybir as mybir
import concourse.tile as tile
from concourse.bass_utils import run_bass_kernel_spmd

F32 = mybir.dt.float32
F32R = mybir.dt.float32r
BF16 = mybir.dt.bfloat16
CDT = BF16          # compute dtype for TensorE-facing tensors
AF = mybir.ActivationFunctionType

B, T, C, H, D = 2, 2048, 1024, 16, 64
TT = B * T            # 4096 (batch folded into time)
N_CORES = 8
IB = 512              # i-block (query columns per score tile)
JT = 128              # j-tile (key rows per score tile)
TBLK = 512            # t-block for projections
N_CB = C // 128       # 8 contraction blocks
N_TB = TT // TBLK     # 8
N_MT = C // 128       # 8 Wproj column tiles
N_IB = T // IB        # 4 i-blocks per batch element
N_JT = T // JT        # 16 j-tiles per batch element


def _plan(P):
    """Classify each (j-tile, i-block) score tile of the [T, T] mask.

    allowed(i, j) = (i < P) | (j < P) | (j <= i)   [negation of the reference's
    disallow = (i >= P) & (j >= P) & (j > i)]

    Returns (plan, patterns): plan[ib] = list of (jt, mask_id_or_None) for
    tiles that must be computed; patterns = float32 [n_pat, JT, IB] of 0/1.
    """
    patterns, pat_ids, plan = [], {}, []
    ii_base = np.arange(IB)[None, :]
    jj_base = np.arange(JT)[:, None]
    for ib in range(N_IB):
        i0 = ib * IB
        row = []
        for jt in range(N_JT):
            j0 = jt * JT
            ii = i0 + ii_base
            jj = j0 + jj_base
            allowed = (ii < P) | (jj < P) | (jj <= ii)
            if not allowed.any():
                continue
            if allowed.all():
                row.append((jt, None))
            elif j0 >= P:
                # allowed = (i < P) | (j <= i): columns ii < P-i0 fully
                # allowed; staircase on the rest -> one gpsimd affine_select
                c0 = max(0, min(IB, P - i0))
                row.append((jt, ("aff", c0, i0 + c0 - j0)))
            else:
                m = allowed.astype(np.float32)
                key = m.tobytes()
                if key not in pat_ids:
                    pat_ids[key] = len(patterns)
                    patterns.append(m)
                row.append((jt, ("mul", pat_ids[key])))
        plan.append(row)
    pats = np.stack(patterns) if patterns else np.zeros((0, JT, IB), np.float32)
    return plan, pats


def build(plan, n_pat, loop_n=1, phases="ABC", biases_zero=True):
    nc = bacc.Bacc()

    xT_d = nc.dram_tensor("xT", [N_TB, 128, N_CB, TBLK], CDT, kind="ExternalInput")
    w3_d = nc.dram_tensor("w3", [C, 384], CDT, kind="ExternalInput")
    wp_d = nc.dram_tensor("wp", [128, C], CDT, kind="ExternalInput")
    bq_d = nc.dram_tensor("bq", [128, 1], F32, kind="ExternalInput")
    bk_d = nc.dram_tensor("bk", [128, 1], F32, kind="ExternalInput")
    bv_d = nc.dram_tensor("bv", [64, 2], F32, kind="ExternalInput")
    bp_d = nc.dram_tensor("bp", [128, N_MT], F32, kind="ExternalInput")
    if n_pat:
        mk_d = nc.dram_tensor("mk", [n_pat, JT, IB], CDT, kind="ExternalInput")
    out_d = nc.dram_tensor("out", [C, TT], CDT, kind="ExternalOutput")

    with tile.TileContext(nc) as tc:
        with (
            tc.tile_pool(name="const", bufs=1) as const,
            tc.tile_pool(name="big", bufs=1) as bigp,
            tc.tile_pool(name="xtp", bufs=2) as xtp,
            tc.tile_pool(name="vtsp", bufs=2) as vtsp,
            tc.tile_pool(name="attp", bufs=6) as attp,
            tc.tile_pool(name="smallp", bufs=3) as smallp,
            tc.tile_pool(name="outp", bufs=2) as outp,
            tc.tile_pool(name="ps_a", bufs=4, space="PSUM") as ps_a,
            tc.tile_pool(name="ps_b", bufs=1, space="PSUM") as ps_b,
        ):
            # ---- constants (loaded once, outside any timing loop) ----
            w3_sb = const.tile([128, N_CB, 384], CDT)
            nc.sync.dma_start(
                out=w3_sb,
                in_=w3_d.rearrange("(cb p) e -> p cb e", p=128),
            )
            wp_sb = const.tile([128, N_MT, 128], CDT)
            nc.sync.dma_start(
                out=wp_sb,
                in_=wp_d.rearrange("p (mt m) -> p mt m", m=128),
            )
            bq_sb = const.tile([128, 1], F32)
            nc.sync.dma_start(out=bq_sb, in_=bq_d[:, :])
            bk_sb = const.tile([128, 1], F32)
            nc.sync.dma_start(out=bk_sb, in_=bk_d[:, :])
            bv_sb = const.tile([64, 2], F32)
            nc.sync.dma_start(out=bv_sb, in_=bv_d[:, :])
            bp_sb = const.tile([128, N_MT], F32)
            nc.sync.dma_start(out=bp_sb, in_=bp_d[:, :])
            if n_pat:
                mk_sb = const.tile([128, n_pat, IB], CDT)
                nc.sync.dma_start(
                    out=mk_sb, in_=mk_d.rearrange("n p q -> p n q")
                )
            # identity for PE transpose (f32r to match the data dtype)
            id_f32 = const.tile([128, 128], F32)
            nc.vector.memset(id_f32, 0.0)
            nc.gpsimd.affine_select(
                out=id_f32,
                in_=id_f32,
                compare_op=mybir.AluOpType.not_equal,
                fill=1.0,
                base=0,
                pattern=[[-1, 128]],
                channel_multiplier=1,
            )
            id_sb = const.tile([128, 128], CDT)
            nc.vector.tensor_copy(out=id_sb, in_=id_f32)

            ones_f32 = const.tile([128, TT // JT, 1], F32)
            nc.vector.memset(ones_f32, 1.0)
            ones65_f32 = const.tile([65, 64], F32)
            nc.vector.memset(ones65_f32, 1.0)
            ones65 = const.tile([65, 64], CDT)
            nc.vector.tensor_copy(out=ones65, in_=ones65_f32)

            qT2 = bigp.tile([128, TT], CDT)
            kT2 = bigp.tile([128, TT], CDT)
            vsb = bigp.tile([128, TT // JT, 130], CDT)
            yT = bigp.tile([128, TT], CDT)

            # ones columns for the [v | 1] denominator trick
            nc.vector.tensor_copy(out=vsb[:, :, 64:65], in_=ones_f32)
            nc.vector.tensor_copy(out=vsb[:, :, 129:130], in_=ones_f32)

            def body(_iv=None):
                do_a = "A" in phases
                do_b = "B" in phases
                do_c = "C" in phases

                # ---- Phase A: QKV projections (transposed layouts) ----
                for tb in range(N_TB if do_a else 0):
                    ts_ = bass.ts(tb, TBLK)
                    xt = xtp.tile([128, N_CB, TBLK], CDT, tag="xt")
                    nc.gpsimd.dma_start(out=xt, in_=xT_d[tb])
                    for part in range(3):  # q, k, v
                        pm = ps_a.tile([128, TBLK], F32, tag="big")
                        for cb in range(N_CB):
                            nc.tensor.matmul(
                                pm,
                                w3_sb[:, cb, bass.ts(part, 128)],
                                xt[:, cb, :],
                                start=(cb == 0),
                                stop=(cb == N_CB - 1),
                            )
                        if part == 0:
                            if biases_zero:
                                nc.vector.tensor_copy(out=qT2[:, ts_], in_=pm)
                            else:
                                nc.scalar.activation(
                                    out=qT2[:, ts_], in_=pm, func=AF.Identity,
                                    bias=bq_sb, scale=0.125,
                                )
                        elif part == 1:
                            if biases_zero:
                                nc.vector.tensor_copy(out=kT2[:, ts_], in_=pm)
                            else:
                                nc.scalar.activation(
                                    out=kT2[:, ts_], in_=pm, func=AF.Identity,
                                    bias=bk_sb, scale=1.0,
                                )
                        else:
                            vts = vtsp.tile([128, TBLK], CDT, tag="vts")
                            nc.vector.tensor_copy(out=vts, in_=pm)
                            for st in range(TBLK // 128):
                                pv = ps_a.tile([128, 128], CDT, tag="big")
                                nc.tensor.transpose(
                                    pv, vts[:, bass.ts(st, 128)], id_sb
                                )
                                jg = tb * (TBLK // 128) + st
                                nc.vector.tensor_copy(
                                    out=vsb[:, jg, 0:64], in_=pv[:, 0:64]
                                )
                                nc.vector.tensor_copy(
                                    out=vsb[:, jg, 65:129], in_=pv[:, 64:128]
                                )

                # ---- Phase B: attention per (batch, i-block), heads paired ----
                def attention_block(b, ib):
                    icols = bass.ds(b * T + ib * IB, IB)
                    tiles = plan[ib]
                    last = len(tiles) - 1
                    ot = [
                        ps_b.tile([65, IB], F32, tag=f"o{h}", name=f"ot{h}",
                                  bufs=2)
                        for h in (0, 1)
                    ]
                    for idx, (jt, mid) in enumerate(tiles):
                        jcols = bass.ds(b * T + jt * JT, JT)
                        sps = []
                        for h in (0, 1):
                            sp = ps_a.tile([128, IB], F32, tag="big")
                            hd = bass.ds(h * 64, 64)
                            nc.tensor.matmul(
                                sp, kT2[hd, jcols], qT2[hd, icols],
                                start=True, stop=True,
                            )
                            sps.append(sp)
                        for h in (0, 1):
                            att = attp.tile([128, IB], CDT, tag="att")
                            nc.scalar.activation(
                                out=att, in_=sps[h], func=AF.Exp, scale=1.0
                            )
                            if mid is not None:
                                if mid[0] == "aff":
                                    _, c0, ab = mid
                                    nc.gpsimd.affine_select(
                                        out=att[:, c0:IB],
                                        in_=att[:, c0:IB],
                                        compare_op=mybir.AluOpType.is_ge,
                                        fill=0.0,
                                        base=ab,
                                        pattern=[[1, IB - c0]],
                                        channel_multiplier=-1,
                                    )
                                else:
                                    nc.vector.tensor_mul(
                                        out=att, in0=att,
                                        in1=mk_sb[:, mid[1], :],
                                    )
                            nc.tensor.matmul(
                                ot[h],
                                vsb[:, b * N_JT + jt, bass.ds(h * 65, 65)],
                                att,
                                start=(idx == 0),
                                stop=(idx == last),
                            )
                    return ot

                def attention_epilogue(b, ib, ot):
                    icols = bass.ds(b * T + ib * IB, IB)
                    for h in (0, 1):
                        lst = smallp.tile([65, IB], CDT, tag="lst")
                        nc.vector.tensor_copy(
                            out=lst[64:65, :], in_=ot[h][64:65, :]
                        )
                        with nc.allow_low_precision(
                            reason="recip of softmax denom at f32r (~1e-4)"
                        ):
                            nc.vector.reciprocal(lst[64:65, :], lst[64:65, :])
                        lb_ps = ps_a.tile([64, IB], F32, tag="big")
                        nc.tensor.matmul(
                            lb_ps, ones65[64:65, :], lst[64:65, :],
                            start=True, stop=True,
                        )
                        lb = smallp.tile([64, IB], F32, tag="lb")
                        nc.vector.tensor_copy(out=lb, in_=lb_ps)
                        if h == 0:
                            nc.vector.tensor_mul(
                                out=yT[0:64, icols], in0=ot[h][0:64, :], in1=lb
                            )
                            if not biases_zero:
                                nc.vector.tensor_scalar_add(
                                    out=yT[0:64, icols],
                                    in0=yT[0:64, icols],
                                    scalar1=bv_sb[:, 0:1],
                                )
                        else:
                            yst = smallp.tile([64, IB], CDT, tag="yst")
                            nc.vector.tensor_mul(
                                out=yst, in0=ot[h][0:64, :], in1=lb
                            )
                            if not biases_zero:
                                nc.vector.tensor_scalar_add(
                                    out=yst, in0=yst, scalar1=bv_sb[:, 1:2]
                                )
                            nc.gpsimd.dma_start(
                                out=yT[64:128, icols], in_=yst
                            )
                def proj_block(b, ib):
                    if not do_c:
                        return
                    icols = bass.ds(b * T + ib * IB, IB)
                    for mt in range(N_MT):
                        pp = ps_a.tile([128, IB], F32, tag="big", name="ppi")
                        nc.tensor.matmul(
                            pp, wp_sb[:, mt, :], yT[:, icols],
                            start=True, stop=True,
                        )
                        osb = outp.tile([128, IB], CDT, tag="osb", name="osbi")
                        if biases_zero:
                            nc.vector.tensor_copy(out=osb, in_=pp)
                        else:
                            nc.vector.tensor_scalar_add(
                                out=osb, in0=pp,
                                scalar1=bp_sb[:, mt : mt + 1],
                            )
                        nc.sync.dma_start(
                            out=out_d[bass.ts(mt, 128), icols], in_=osb
                        )

                prev = None
                for b in range(B if do_b else 0):
                    for ib in range(N_IB):
                        ot = attention_block(b, ib)
                        if prev is not None:
                            attention_epilogue(*prev)
                        prev = (b, ib, ot)
                if prev is not None:
                    attention_epilogue(*prev)
                # ---- Phase C: output projection, standalone mt-major ----
                for mt in range(N_MT if do_c else 0):
                    osb = outp.tile([128, TT], CDT, tag="osb")
                    for tb in range(N_TB):
                        ts_ = bass.ts(tb, TBLK)
                        pp = ps_a.tile([128, TBLK], F32, tag="big")
                        nc.tensor.matmul(
                            pp, wp_sb[:, mt, :], yT[:, ts_],
                            start=True, stop=True,
                        )
                        if biases_zero:
                            if (mt + tb) % 2 == 0:
                                nc.vector.tensor_copy(out=osb[:, ts_], in_=pp)
                            else:
                                nc.scalar.activation(
                                    out=osb[:, ts_], in_=pp, func=AF.Copy
                                )
                        else:
                            nc.vector.tensor_scalar_add(
                                out=osb[:, ts_], in0=pp,
                                scalar1=bp_sb[:, mt : mt + 1],
                            )
                    nc.sync.dma_start(out=out_d[bass.ts(mt, 128), :], in_=osb)

            if loop_n == 1:
                body()
            else:
                with tc.For_i(0, loop_n, 1) as iv:
                    body(iv)

    nc.finalize()
    return nc


_CACHE = {}


def _get_nc(P, loop_n=1, phases="ABC", biases_zero=True):
    key = (int(P), int(loop_n), phases, biases_zero)
    if key not in _CACHE:
        plan, pats = _plan(int(P))
        _CACHE[key] = (
            build(plan, len(pats), loop_n, phases, biases_zero), pats,
        )
    return _CACHE[key]


def make_in_maps(x, Wqkv, bqkv, Wproj, bproj, pats):
    x = np.asarray(x, np.float32)
    Wqkv = np.asarray(Wqkv, np.float32)
    bqkv = np.asarray(bqkv, np.float32)
    Wproj = np.asarray(Wproj, np.float32)
    bproj = np.asarray(bproj, np.float32)
    xT = x.reshape(TT, C).T
    # X2[tb, p, cb, t] = xT[cb*128+p, tb*512+t] -> contiguous per-tile DMA
    import ml_dtypes
    xTt = np.ascontiguousarray(
        xT.reshape(N_CB, 128, N_TB, TBLK).transpose(2, 1, 0, 3)
    ).astype(ml_dtypes.bfloat16)
    biases_zero = not (bqkv.any() or bproj.any())
    in_maps = []
    for i in range(N_CORES):
        cs = slice(128 * i, 128 * i + 128)
        wq = Wqkv[:, cs]
        if biases_zero:
            wq = wq / 8.0
        w3 = np.ascontiguousarray(
            np.concatenate(
                [wq, Wqkv[:, 1024:2048][:, cs], Wqkv[:, 2048:3072][:, cs]],
                axis=1,
            )
        ).astype(ml_dtypes.bfloat16)
        m = {
            "xT": xTt,
            "w3": w3,
            "wp": np.ascontiguousarray(Wproj[cs, :]).astype(ml_dtypes.bfloat16),
            "bq": np.ascontiguousarray((bqkv[0:1024][cs] / 8.0).reshape(128, 1)),
            # (q weights pre-scaled by 1/8 on host when biases are zero)
            "bk": np.ascontiguousarray(bqkv[1024:2048][cs].reshape(128, 1)),
            "bv": np.ascontiguousarray(
                bqkv[2048:3072][cs].reshape(2, 64).T
            ),
            "bp": (
                np.ascontiguousarray(bproj.reshape(N_MT, 128).T)
                if i == 0
                else np.zeros((128, N_MT), np.float32)
            ),
        }
        if len(pats):
            m["mk"] = pats.astype(ml_dtypes.bfloat16)
        in_maps.append(m)
    return in_maps


def kernel(x, Wqkv, bqkv, Wproj, bproj, prefix_len):
    P = int(prefix_len)
    bz = not (np.asarray(bqkv).any() or np.asarray(bproj).any())
    nc, pats = _get_nc(P, 1, "ABC", bz)
    in_maps = make_in_maps(x, Wqkv, bqkv, Wproj, bproj, pats)
    res = run_bass_kernel_spmd(nc, in_maps, core_ids=list(range(N_CORES)))
    acc = np.zeros((C, TT), np.float32)
    for i in range(N_CORES):
        acc += res.results[i]["out"].astype(np.float32)
    return np.ascontiguousarray(acc.T).reshape(B, T, C)


def make_runner(nc):
    """Build a reusable jitted 8-core runner for `nc` (compiles once)."""
    import jax
    from jax.sharding import Mesh, PartitionSpec
    from jax.experimental.shard_map import shard_map
    from concourse import bass2jax

    bass2jax.install_neuronx_cc_hook()
    partition_name = nc.partition_id_tensor.name if nc.partition_id_tensor else None
    in_names, out_names, out_avals, zero_outs = [], [], [], []
    for alloc in nc.m.functions[0].allocations:
        if not isinstance(alloc, mybir.MemoryLocationSet):
            continue
        name = alloc.memorylocations[0].name
        if alloc.kind == "ExternalInput":
            if name != partition_name:
                in_names.append(name)
        elif alloc.kind == "ExternalOutput":
            out_names.append(name)
            shape = tuple(alloc.tensor_shape)
            dtype = mybir.dt.np(alloc.dtype)
            out_avals.append(jax.core.ShapedArray(shape, dtype))
            zero_outs.append(np.zeros(shape, dtype))
    n_params = len(in_names)
    all_names = in_names + out_names + ([partition_name] if partition_name else [])

    def _body(*args):
        operands = list(args)
        if partition_name is not None:
            operands.append(bass2jax.partition_id_tensor())
        return tuple(bass2jax._bass_exec_p.bind(
            *operands, out_avals=tuple(out_avals), in_names=tuple(all_names),
            out_names=tuple(out_names), lowering_input_output_aliases=(),
            sim_require_finite=True, sim_require_nnan=True, nc=nc))

    devices = jax.devices()[:N_CORES]
    mesh = Mesh(np.asarray(devices), ("core",))
    nin = n_params + len(out_names)
    sharded = jax.jit(
        shard_map(_body, mesh=mesh, in_specs=(PartitionSpec("core"),) * nin,
                  out_specs=(PartitionSpec("core"),) * len(out_names),
                  check_rep=False),
        keep_unused=True)

    from jax.sharding import NamedSharding

    def prepare(in_maps):
        concat_in = [
            np.concatenate([np.asarray(in_maps[c][n]) for c in range(N_CORES)], axis=0)
            for n in in_names
        ]
        concat_zeros = [
            np.zeros((N_CORES * z.shape[0], *z.shape[1:]), z.dtype) for z in zero_outs
        ]
        sh = NamedSharding(mesh, PartitionSpec("core"))
        return [jax.device_put(a, sh) for a in concat_in + concat_zeros]

    def run_device(args):
        out_arrs = sharded(*args)
        jax.block_until_ready(out_arrs)
        return out_arrs

    def fetch(out_arrs):
        return [
            {name: np.asarray(out_arrs[i]).reshape(N_CORES, *out_avals[i].shape)[c]
             for i, name in enumerate(out_names)}
            for c in range(N_CORES)
        ]

    def run(in_maps):
        return fetch(run_device(prepare(in_maps)))

    run.prepare = prepare
    run.run_device = run_device
    run.fetch = fetch
    return run
